# revision 26
# baseline (speedup 1.0000x reference)
"""Trainium2 Bass kernel for nn_Binary_CNN2 (binarized CNN, eval mode).

Data-parallel over 8 NeuronCores: batch 4096 -> 512 per core.

Per-core pipeline:
  x [512,1,28,28] f32
  -> sign (+-0.5, bf16) [DVE]
  -> DMA-transpose to padded DRAM layout xpad[i' (34), j' (32), b] (zero borders)
  -> slab reload: SBUF [34 rows, (j',b)] contiguous
  -> im2col via SBUF->SBUF DMAs (long contiguous runs) -> rhs [36,(r,j,b)]
  -> conv as block-diag matmul K=36 (3dx*3dy*4 row-groups) -> psum [(g,o), cols]
  -> 2x2 maxpool + threshold-sign, split across DVE-reduce and ACT-sign paths
     -> a [128=(g,o), 4=ilp, 14=jp, 512=b] fp8 {+-1}
  -> FC1: z1.T[h,b] = sum W2b.T @ a  (fp8 DoubleRow, exact int accum in PSUM)
  -> BN2 affine (ACT scale/bias) + clip (DVE) -> z.T [128=h,16=ht,512=b] f32
  -> FC2: logits[b,10] fp32 + b3, log_softmax -> out [512, 10] f32
"""

import numpy as np
import ml_dtypes

import concourse.bass as bass
import concourse.mybir as mybir
import concourse.tile as tile
from concourse import bacc
from concourse.masks import make_identity
from concourse.bass_utils import run_bass_kernel_spmd

EPS = 1e-5
NCORES = 8
B = 512          # batch per core
BH = 256         # batch half (conv matmul free dim)
H = 2048
C = 10
F32 = mybir.dt.float32
BF16 = mybir.dt.bfloat16
FP8 = mybir.dt.float8e4

# conv row-groups over the 28 image rows: sizes 8,8,8,4 (pool-pair aligned)
# valid pooled-row-pair indices per group: g<3 -> ilp 0..3, g=3 -> ilp 0..1
NPART_FOR_ILP = [128, 128, 96, 96]  # FC1 contraction rows valid per ilp

SIMPLIFY = set()


def _f(c, k):
    """FC1 feature index map: chunk c=(ilp*14+jp), row k=(g*32+o) -> flat f."""
    ilp, jp = divmod(c, 14)
    g, o = divmod(k, 32)
    if g < 3:
        ip = 4 * g + ilp
    else:
        if ilp >= 2:
            return None
        ip = 12 + ilp
    return o * 196 + ip * 14 + jp


def build_nc(loop_n=None, parts=("s0", "conv", "fc1", "fc2"), simplify=None):
    simplify = SIMPLIFY if simplify is None else set(simplify)
    nc = bacc.Bacc("TRN2", target_bir_lowering=False, debug=False,
                   num_devices=NCORES)

    xin = nc.dram_tensor("x", [B, 28 * 28], F32, kind="ExternalInput")
    wc = nc.dram_tensor("wc", [36, 128], FP8, kind="ExternalInput")
    negt1 = nc.dram_tensor("negt1", [128, 1], F32, kind="ExternalInput")
    w2b = nc.dram_tensor("w2b", [16, 128, 56, 128], FP8, kind="ExternalInput")
    s2t = nc.dram_tensor("s2t", [128, 16], F32, kind="ExternalInput")
    t2t = nc.dram_tensor("t2t", [128, 16], F32, kind="ExternalInput")
    w3t = nc.dram_tensor("w3t", [16, 128, C], F32, kind="ExternalInput")
    b3r = nc.dram_tensor("b3r", [128, C], F32, kind="ExternalInput")
    out = nc.dram_tensor("out", [B, C], F32, kind="ExternalOutput")

    # padded transposed image: xpad[i' (34 incl 4 slack), j' (32), b] bf16
    xpad = nc.dram_tensor("xpad", [34 * 32 * B], BF16, kind="Internal")

    hw_q = [nc.sync, nc.scalar]  # two HWDGE issue queues

    with tile.TileContext(nc) as tc:
        with (
            tc.tile_pool(name="consts", bufs=1) as consts,
            tc.tile_pool(name="persist", bufs=1) as persist,
        ):
            # ---- constants to SBUF (outside any timing loop) ----
            wc_sb = consts.tile([36, 128], FP8)
            nc.sync.dma_start(wc_sb[:], wc.ap())
            negt1_sb = consts.tile([128, 1], F32)
            nc.sync.dma_start(negt1_sb[:], negt1.ap())
            s2_sb = consts.tile([128, 16], F32)
            nc.sync.dma_start(s2_sb[:], s2t.ap())
            t2_sb = consts.tile([128, 16], F32)
            nc.sync.dma_start(t2_sb[:], t2t.ap())
            w3_sb = consts.tile([128, 16, C], F32)
            nc.sync.dma_start(w3_sb[:], w3t.ap().rearrange("t p c -> p t c"))
            b3_sb = consts.tile([128, C], F32)
            nc.sync.dma_start(b3_sb[:], b3r.ap())

            a_sb = persist.tile([128, 4, 14, B], FP8)       # {+-1}
            ident = consts.tile([128, 128], BF16)
            make_identity(nc, ident[:])

            def _body_s0():
              with tc.tile_pool(name="stage0", bufs=1) as s0:
                x_sb = s0.tile([128, 4, 28 * 28], F32, tag="x")
                nc.sync.dma_start(
                    x_sb[:], xin.ap().rearrange("(bo p) f -> p bo f", p=128))
                xb_sb = s0.tile([128, 4, 28, 32], BF16, tag="xb")
                nc.vector.memset(xb_sb[:], 0.0)
                # sign: (x >= 0) - 0.5 -> {+0.5, -0.5}; conv weights carry x2
                nc.vector.tensor_scalar(
                    xb_sb[:, :, :, 0:28],
                    x_sb[:].rearrange("p bo (h w) -> p bo h w", h=28),
                    0.0, 0.5, mybir.AluOpType.is_ge, mybir.AluOpType.subtract)

                # zero the whole xpad buffer (borders stay 0)
                zeros_sb = s0.tile([128, 1088], BF16, tag="zeros")
                nc.vector.memset(zeros_sb[:], 0.0)
                for q in range(4):
                    nc.gpsimd.dma_start(
                        bass.AP(xpad, q * 128 * 1088,
                                [[1088, 128], [1, 1088]]),
                        zeros_sb[:])

                # transpose b <-> (i,j32) in 128x128 tiles on the (idle)
                # TensorE, evacuating PSUM via ACT/DVE copies
                xT_sb = s0.tile([128, 7, 4, 128], BF16, tag="xT")
                with tc.tile_pool(name="tpsum", bufs=4, space="PSUM") as tps:
                    for c in range(7):
                        for bo in range(4):
                            src = xb_sb[:, bo].rearrange("p h w -> p (h w)")
                            pst = tps.tile([128, 128], BF16, tag="t")
                            nc.tensor.transpose(
                                pst[:], src[:, c * 128:(c + 1) * 128],
                                ident[:])
                            if (c * 4 + bo) % 2 == 0:
                                nc.scalar.copy(xT_sb[:, c, bo, :], pst[:])
                            else:
                                nc.vector.tensor_copy(xT_sb[:, c, bo, :],
                                                      pst[:])
                # write interior of xpad at element offset 33*512
                # dst(q,c,bo,bl) = (c*128+q)*512 + 33*512 + bo*128 + bl
                nc.gpsimd.dma_start(
                    bass.AP(xpad, 33 * B,
                            [[B, 128], [128 * B, 7], [128, 4], [1, 128]]),
                    xT_sb[:])

            def _body_conv():
              with (
                  tc.tile_pool(name="im2col", bufs=3) as imp,
                  tc.tile_pool(name="ptmp", bufs=4) as ptmp,
                  tc.tile_pool(name="cpsum", bufs=3, space="PSUM") as cpsum,
              ):
                for lam in range(4):          # pooled-row-pair index (ilp)
                    rhs_t = imp.tile([36, 2, 28, B], FP8, tag="rhs")
                    if "fastim2col" in simplify:
                        nc.gpsimd.dma_start(
                            rhs_t[:],
                            bass.AP(xpad, 0, [[766, 36], [1, 2 * 28 * B]]))
                    else:
                        # one SWDGE cast-DMA (bf16->fp8) per (dy,dx):
                        # [4 g-rows, 2 r-rows, 28*512 contiguous (j,b)]
                        for dy in range(3):
                            for dx in range(3):
                                p0 = dx * 12 + dy * 4
                                off = (2 * lam + dy) * 32 * B + dx * B
                                srcap = bass.AP(
                                    xpad, off,
                                    [[8 * 32 * B, 4], [32 * B, 2],
                                     [1, 28 * B]])
                                nc.gpsimd.dma_start(rhs_t[p0:p0 + 4], srcap)
                    for bh in range(2):       # batch half
                        for jp in range(14):
                            # slot = s*2 + r: each pool pair spans both banks
                            psq = cpsum.tile([128, 4, BH], F32, tag="cq")
                            for r in range(2):
                                for s in range(2):
                                    nc.tensor.matmul(
                                        psq[:, s * 2 + r, :],
                                        wc_sb[:],
                                        rhs_t[:, r, 2 * jp + s,
                                              bh * BH:(bh + 1) * BH],
                                        start=True, stop=True)
                            if "noepi" in simplify:
                                continue
                            a_slice = a_sb[:, lam, jp, bh * BH:(bh + 1) * BH]
                            if jp < 5:
                                # pathway A: DVE strided max-reduce, ACT sign
                                pm = ptmp.tile([128, BH], BF16, tag="pm")
                                nc.vector.tensor_reduce(
                                    pm[:],
                                    psq[:].rearrange("p s b -> p b s"),
                                    axis=mybir.AxisListType.X,
                                    op=mybir.AluOpType.max)
                                nc.scalar.activation(
                                    a_slice, pm[:],
                                    mybir.ActivationFunctionType.Sign,
                                    bias=negt1_sb[:])
                            else:
                                # pathway B: one big ACT sign, DVE bf16 maxes
                                sq = ptmp.tile([128, 4, BH], BF16, tag="sq")
                                nc.scalar.activation(
                                    sq[:], psq[:],
                                    mybir.ActivationFunctionType.Sign,
                                    bias=negt1_sb[:])
                                m1 = ptmp.tile([128, 2, BH], BF16, tag="m1")
                                nc.vector.tensor_tensor(
                                    m1[:, 0, :], sq[:, 0, :], sq[:, 1, :],
                                    mybir.AluOpType.max)
                                nc.vector.tensor_tensor(
                                    m1[:, 1, :], sq[:, 2, :], sq[:, 3, :],
                                    mybir.AluOpType.max)
                                nc.vector.tensor_tensor(
                                    a_slice, m1[:, 0, :], m1[:, 1, :],
                                    mybir.AluOpType.max)

            def _body_fc1(zt_sb):
              with (
                  tc.tile_pool(name="w2pool", bufs=3) as w2p,
                  tc.tile_pool(name="zpsum", bufs=2, space="PSUM") as zps,
              ):
                for ht in range(16):
                    w2_sb = w2p.tile([128, 56, 128], FP8, tag="w2")
                    nc.sync.dma_start(w2_sb[:], w2b.ap()[ht])
                    psz = zps.tile([128, B], F32, tag="z")
                    for cp in range(28):
                        lam, jph = divmod(cp, 7)
                        jp = 2 * jph
                        c = lam * 14 + jp
                        kk = NPART_FOR_ILP[lam]
                        nc.tensor.matmul(
                            psz[:],
                            w2_sb[0:kk, c:c + 2, :],
                            a_sb[0:kk, lam, jp:jp + 2, :],
                            start=(cp == 0), stop=(cp == 27),
                            perf_mode=mybir.MatmulPerfMode.DoubleRow)
                    nc.scalar.activation(
                        zt_sb[:, ht, :], psz[:],
                        mybir.ActivationFunctionType.Identity,
                        bias=t2_sb[:, ht:ht + 1],
                        scale=s2_sb[:, ht:ht + 1])
                    nc.vector.tensor_scalar(
                        zt_sb[:, ht, :], zt_sb[:, ht, :],
                        1.0, -1.0, mybir.AluOpType.min, mybir.AluOpType.max)

            def _body_fc2(zt_sb, out_sb):
              with (
                  tc.tile_pool(name="cctmp", bufs=2) as cct,
                  tc.tile_pool(name="lpsum", bufs=1, space="PSUM") as lps,
              ):
                for bt in range(4):
                    psl = lps.tile([128, C], F32, tag="l")
                    for ht in range(16):
                        nc.tensor.matmul(
                            psl[:],
                            zt_sb[:, ht, bt * 128:(bt + 1) * 128],
                            w3_sb[:, ht, :],
                            start=(ht == 0), stop=(ht == 15))
                    lg = cct.tile([128, C], F32, tag="lg")
                    nc.vector.tensor_add(lg[:], psl[:], b3_sb[:])
                    m = cct.tile([128, 1], F32, tag="m")
                    nc.vector.reduce_max(m[:], lg[:],
                                         axis=mybir.AxisListType.X)
                    negm = cct.tile([128, 1], F32, tag="negm")
                    nc.vector.tensor_scalar_mul(negm[:], m[:], -1.0)
                    e = cct.tile([128, C], F32, tag="e")
                    nc.scalar.activation(
                        e[:], lg[:], mybir.ActivationFunctionType.Exp,
                        bias=negm[:])
                    se = cct.tile([128, 1], F32, tag="se")
                    nc.vector.reduce_sum(se[:], e[:],
                                         axis=mybir.AxisListType.X)
                    lns = cct.tile([128, 1], F32, tag="lns")
                    nc.scalar.activation(
                        lns[:], se[:], mybir.ActivationFunctionType.Ln)
                    tot = cct.tile([128, 1], F32, tag="tot")
                    nc.vector.tensor_add(tot[:], m[:], lns[:])
                    nc.vector.tensor_scalar(
                        out_sb[:, bt, :], lg[:], tot[:], None,
                        mybir.AluOpType.subtract)

            def body():
                if "s0" in parts:
                    _body_s0()
                if "conv" in parts:
                    _body_conv()
                    if "noepi" in simplify:
                        nc.gpsimd.memset(a_sb[:], 1.0)
                elif "fc1" in parts:
                    nc.vector.memset(a_sb[:], 1.0)  # ablation filler
                with tc.tile_pool(name="late", bufs=1) as late:
                    zt_sb = late.tile([128, 16, B], F32)
                    out_sb = late.tile([128, 4, C], F32)
                    if "fc1" in parts:
                        _body_fc1(zt_sb)
                    elif "fc2" in parts:
                        nc.vector.memset(zt_sb[:], 0.5)  # ablation filler
                    if "fc2" in parts:
                        _body_fc2(zt_sb, out_sb)
                    else:
                        nc.vector.memset(out_sb[:], 0.0)
                    nc.sync.dma_start(
                        out.ap().rearrange("(bo p) c -> p bo c", p=128),
                        out_sb[:])

            if loop_n is None:
                body()
            else:
                with tc.For_i(0, loop_n, 1):
                    body()

    nc.finalize()
    return nc


_NC_CACHE = {}


def _get_nc(loop_n=None, parts=("s0", "conv", "fc1", "fc2")):
    key = (loop_n, tuple(parts), tuple(sorted(SIMPLIFY)))
    if key not in _NC_CACHE:
        _NC_CACHE[key] = build_nc(loop_n, parts)
    return _NC_CACHE[key]


def _host_prep(W1, b1, g1, be1, m1, v1, W2, b2, g2, be2, m2, v2, W3, b3):
    """Precompute small device-side constant tensors (numpy, f32)."""
    s1 = (g1 / np.sqrt(v1 + EPS)).astype(np.float32)
    assert np.all(s1 != 0)
    # bn1 >= 0  <=>  sign(conv_nb - t1[o]) == sign(s1[o]); fold sign(s1)
    # into W2's columns so the device only computes sign(conv_nb - t1)
    t1 = (m1 - be1 / s1 - b1).astype(np.float32)
    sgn1 = np.where(s1 >= 0, 1.0, -1.0).astype(np.float32)
    negt1 = np.repeat(-t1[None, :], 4, axis=0).reshape(128, 1)

    wc = np.zeros((36, 128), np.float32)
    w1s = np.where(W1[:, 0] >= 0, 2.0, -2.0).astype(np.float32)  # [32,3,3] x2
    for dy in range(3):
        for dx in range(3):
            for g in range(4):
                p = dx * 12 + dy * 4 + g
                wc[p, g * 32:(g + 1) * 32] = w1s[:, dy, dx]
    wc = wc.astype(ml_dtypes.float8_e4m3)

    w2s = np.where(W2 >= 0, 1.0, -1.0).astype(np.float32)  # [H, F1]
    w2s = w2s * sgn1[np.arange(w2s.shape[1]) // 196][None, :]
    w2bp = np.zeros((16, 128, 56, 128), np.float32)  # [ht, k, c, hh]
    for c in range(56):
        ilp, jp = divmod(c, 14)
        for g in range(4):
            if _f(c, g * 32) is None:
                continue
            ip = 4 * g + ilp if g < 3 else 12 + ilp
            fs = np.arange(32) * 196 + ip * 14 + jp  # f for o=0..31
            # w2bp[ht, g*32+o, c, hh] = w2s[ht*128+hh, fs[o]]
            blk = w2s[:, fs].reshape(16, 128, 32)   # [ht, hh, o]
            w2bp[:, g * 32:(g + 1) * 32, c, :] = blk.transpose(0, 2, 1)
    w2bp = w2bp.astype(ml_dtypes.float8_e4m3)

    s2 = (g2 / np.sqrt(v2 + EPS)).astype(np.float32)
    t2 = (be2 + s2 * (b2 - m2)).astype(np.float32)
    s2t = s2.reshape(16, 128).T.copy()
    t2t = t2.reshape(16, 128).T.copy()

    w3t = np.ascontiguousarray(W3.T.astype(np.float32)).reshape(16, 128, C)
    b3r = np.repeat(b3[None, :].astype(np.float32), 128, axis=0)
    return dict(wc=wc, negt1=negt1, w2b=w2bp, s2t=s2t, t2t=t2t,
                w3t=w3t, b3r=np.ascontiguousarray(b3r))


def _make_in_maps(x, consts):
    xs = np.asarray(x, np.float32).reshape(NCORES, B, 28 * 28)
    in_maps = []
    for i in range(NCORES):
        m = {"x": np.ascontiguousarray(xs[i])}
        m.update(consts)
        in_maps.append(m)
    return in_maps


def _prep_all(inputs):
    names = ["W1", "b1", "g1", "be1", "m1", "v1", "W2", "b2", "g2", "be2",
             "m2", "v2", "W3", "b3"]
    return _host_prep(*[np.asarray(inputs[n], np.float32) for n in names])


def kernel(x, **weights):
    consts = _prep_all(weights)
    nc = _get_nc(None)
    in_maps = _make_in_maps(x, consts)
    res = run_bass_kernel_spmd(nc, in_maps, core_ids=list(range(NCORES)))
    outs = [res.results[i]["out"] for i in range(NCORES)]
    return np.concatenate(outs, axis=0).astype(np.float32)


def _make_runner(nc, in_maps):
    """Build a reusable executor with inputs resident on device (no re-upload)."""
    import jax
    import jax.numpy as jnp
    from jax.sharding import Mesh, PartitionSpec, NamedSharding
    from jax.experimental.shard_map import shard_map
    from concourse import bass2jax
    from concourse.bass2jax import _bass_exec_p, install_neuronx_cc_hook

    install_neuronx_cc_hook()
    n_cores = len(in_maps)
    partition_name = nc.partition_id_tensor.name if nc.partition_id_tensor else None
    in_names, out_names, out_avals, zero_outs = [], [], [], []
    for alloc in nc.m.functions[0].allocations:
        if not isinstance(alloc, mybir.MemoryLocationSet):
            continue
        name = alloc.memorylocations[0].name
        if alloc.kind == "ExternalInput":
            if name != partition_name:
                in_names.append(name)
        elif alloc.kind == "ExternalOutput":
            shape = tuple(alloc.tensor_shape)
            dtype = mybir.dt.np(alloc.dtype)
            out_names.append(name)
            out_avals.append(jax.core.ShapedArray(shape, dtype))
            zero_outs.append(np.zeros(shape, dtype))
    n_params = len(in_names)
    n_outs = len(out_avals)
    in_names.extend(out_names)
    if partition_name is not None:
        in_names.append(partition_name)
    donate = tuple(range(n_params, n_params + n_outs))

    def _body(*args):
        operands = list(args)
        if partition_name is not None:
            operands.append(bass2jax.partition_id_tensor())
        outs = _bass_exec_p.bind(
            *operands, out_avals=tuple(out_avals), in_names=tuple(in_names),
            out_names=tuple(out_names), lowering_input_output_aliases=(),
            sim_require_finite=True, sim_require_nnan=True, nc=nc)
        return tuple(outs)

    devices = jax.devices()[:n_cores]
    mesh = Mesh(np.asarray(devices), ("core",))
    sharded = jax.jit(
        shard_map(_body, mesh=mesh,
                  in_specs=(PartitionSpec("core"),) * (n_params + n_outs),
                  out_specs=(PartitionSpec("core"),) * n_outs,
                  check_rep=False),
        donate_argnums=donate, keep_unused=True)
    shard = NamedSharding(mesh, PartitionSpec("core"))
    per_core = [[np.asarray(m[nm]) for nm in in_names[:n_params]]
                for m in in_maps]
    dev_in = [jax.device_put(
                np.concatenate([per_core[c][i] for c in range(n_cores)],
                               axis=0), shard)
              for i in range(n_params)]
    concat_zero_shapes = [((n_cores * z.shape[0],) + z.shape[1:], z.dtype)
                          for z in zero_outs]

    def run():
        zeros = [jnp.zeros(s, d, device=shard) for s, d in concat_zero_shapes]
        outs = sharded(*dev_in, *zeros)
        jax.block_until_ready(outs)
        return outs

    return run


def measure_exec_ns(inputs, n_lo=4, n_hi=132, reps=11):
    """HW exec time per pipeline iteration via looped-kernel wall-clock delta."""
    import time
    consts = _prep_all(inputs)
    in_maps = _make_in_maps(inputs["x"], consts)

    def med_time(loop_n):
        nc = _get_nc(loop_n, measure_exec_ns.parts)
        run = _make_runner(nc, in_maps)
        run()  # compile + warm
        ts = []
        for _ in range(reps):
            t0 = time.time()
            run()
            ts.append(time.time() - t0)
        ts.sort()
        return ts[len(ts) // 2], ts

    t_lo, all_lo = med_time(n_lo)
    t_hi, all_hi = med_time(n_hi)
    measure_exec_ns.last = (all_lo, all_hi)
    return (t_hi - t_lo) / (n_hi - n_lo) * 1e9


measure_exec_ns.parts = ("s0", "conv", "fc1", "fc2")
build_nc_looped = build_nc  # marker for test.py
